# revision 21
# baseline (speedup 1.0000x reference)
"""Additive attention (B=4, Q=KV=512, H=256) on 8 Trainium2 NeuronCores.

Math (per batch b):
  q = queries @ W_q            (Q, H)
  k = keys    @ W_k            (KV, H)
  scores[i,j] = sum_h w_v[h] * tanh(q[i,h] + k[j,h])
  attn = softmax_j(scores masked to j < valid_lens[b])
  out  = attn @ values         (Q, V)

Strategy: the baseline evaluated tanh pointwise on the Scalar engine --
64 * sum_b jmax_b * 256 ~ 17M activations/core at 0.83ns/lane-elem puts a
~90us floor on ACT alone.  This kernel instead uses a SEPARABLE expansion:

  tanh(q+k) ~ sum_p [alpha_p * Qf_p(q) + beta_p] * Kf_p(k)      (P=10 pairs)

where Qf/Kf are products of low-harmonic sines of q and k (sin(n*w*(q+k))
splits as s_n(q)c_n(k) + c_n(q)s_n(k)).  The h-reduction then becomes a
plain PE matmul with contraction dim (pair, h).  Feature construction:
3 ACT sine passes per side (sin(w/2 x), sin(w x), sin(2w x); all inputs
within the ACT Sin table range) + a short chain of DVE fp16 products
(cos via sin^2, doubling identities, dirty shifts absorbed by the fit /
per-pair affine).  sign(w_v) is folded into W_q/W_k host-side (tanh odd),
|w_v| and alpha_p fold into one tensor_scalar pass per (pair, h-half);
beta_p * |w_v| contributions are rank-1 over i and ride the softmax-exp
bias via trivial 1-column PE matmuls.  Fit is a 2-D weighted LS of the
actual device feature functions; e2e error ~8.6e-3 (gate 2e-2).

Sharding: batch b -> cores {2b, 2b+1}, 256 query rows each.  All key
windows padded to JW = ceil(max valid/8*8) so the SPMD program is uniform;
padded columns are masked (-1e6) and value rows zeroed.

Layout: h on partitions (2 chunks of 128).  scoresT[j, i] in PSUM per
128-wide j-chunk; softmax in the transposed layout exactly like the
baseline epilogue (exp with additive mask+beta bias, ones-matmul row sums,
reciprocal, transpose-broadcast, values matmul as eT lhsT, per-partition
1/sum scale on the output rows).
"""

import sys
import types

import numpy as np

NEG = -1.0e6
NCORES = 8
TRACE = False  # test.py flips this to get a profiled run
LAST_RESULT = None  # BassKernelResults stash for test.py

# --- fitted expansion: tanh(x+y) ~ sum_p (alpha_p Qf_p(x) + beta_p) Kf_p(y)
# feature slots (same chain both sides):
#   sh=sin(.5wx) s1=sin(wx) s2=sin(2wx) c1d=sh^2 c2d=s1^2 c2t=c2d-.5
#   c4d2=c2t^2 s4t=s2*c2t c4t=c4d2-.125 s8t=s4t*c4t c8d=c4t^2
#   mixQ: c2s4=c2t*s4t s2c4=s2*c4t   mixK: c2c4=c2t*c4t s2s4=s2*s4t
FIT_W = 0.36
PAIRS = [  # (q_feature, k_feature)
    ("s1", "c1d"), ("c1d", "s1"), ("s2", "c2t"), ("c2t", "s2"),
    ("s4t", "c4t"), ("c4t", "s4t"), ("s8t", "c8d"), ("c8d", "s8t"),
    ("c2s4", "c2c4"), ("s2c4", "s2s4"),
]
ALPHAS = [-0.439680893, -4.32525681, -0.845301755, 0.0239388354,
          -5.14401459, -7.71941257, -175.582672, -319.866805,
          -10.9101526, 2.72429164]
BETAS = [0.0, 1.1382438, 0.0, 0.195903978, 0.0, 0.0444884607,
         0.0, 2.46151355, 0.0, 0.0]


def _install_axon_profile_hook():
    """antenv.axon_hooks is missing from this image; concourse needs it for
    trace=True under axon. Register the ctypes-based NTFF hook manually."""
    import antenv

    if "antenv.axon_hooks" in sys.modules:
        return
    m = types.ModuleType("antenv.axon_hooks")
    m._hook = None

    def _set(h):
        m._hook = h

    def _get():
        return m._hook

    m.set_axon_ntff_profile_hook = _set
    m.get_axon_ntff_profile_hook = _get
    sys.modules["antenv.axon_hooks"] = m
    antenv.axon_hooks = m
    try:
        from trn_agent_boot.trn_boot import _ntff_profile_via_ctypes

        m.set_axon_ntff_profile_hook(
            _ntff_profile_via_ctypes("/opt/axon/libaxon_pjrt.so")
        )
    except Exception:
        pass


def _patch_tile_drain():
    """The walrus build in this image allows at most ONE sync-wait command
    per instruction; Tile's kernel-tail drain carries every vector-clock
    wait on a single drain. Split them across a chain of drains."""
    import concourse.mybir as mybir
    import concourse.tile as tile
    from concourse.vector_clock import ScopedClock

    if getattr(tile.TileContext, "_drain_patched", False):
        return

    def _drain_and_barrier_chunked(self, tick_clock, wait_clock):
        d0 = self.nc.sync.drain()
        wait_clock.add_sem_waits(d0.ins, ScopedClock({None: tick_clock.global_clock}))
        si = d0.ins.sync_info
        waits = list(si.on_wait) if si is not None else []
        if len(waits) > 1:
            engs = [
                mybir.EngineType.SP,
                mybir.EngineType.DVE,
                mybir.EngineType.Activation,
                mybir.EngineType.PE,
                mybir.EngineType.Pool,
            ]
            d0.ins.sync_info = mybir.SyncInfo(
                on_wait=waits[:1], on_update=list(si.on_update)
            )
            for i in range(1, len(waits)):
                ev = mybir.InstEventSemaphore(
                    name=f"tail-wait-{i}",
                    engine=engs[i % len(engs)],
                    ins=[],
                    outs=[],
                    sync_info=mybir.SyncInfo(on_wait=[waits[i]], on_update=[]),
                )
                self.nc.register_instruction(ev)
                self.nc.cur_bb.bb.add_instruction(ev)

        self.nc.all_engine_barrier()
        assert self.sems is not None
        popped = self.nc._tile_sem_poison_stack.pop()
        assert popped is self._sem_poison
        self.nc.clear_and_free_semaphores(list(self.sems.allocated().values()))
        self.nc.all_engine_barrier()

    tile.TileContext._drain_and_barrier = _drain_and_barrier_chunked
    tile.TileContext._drain_patched = True


def _split_multi_waits(nc):
    """walrus here allows one sync-wait command per instruction; move extra
    waits onto standalone EventSemaphore instructions."""
    import concourse.mybir as mybir

    n = 0
    for fn in nc.m.functions:
        for blk in fn.blocks:
            out = []
            for inst in blk.instructions:
                si = inst.sync_info
                waits = list(si.on_wait) if si is not None else []
                if len(waits) > 1:
                    for k, w in enumerate(waits[:-1]):
                        ev = mybir.InstEventSemaphore(
                            name=f"{inst.name}-xw{k}",
                            engine=inst.engine,
                            ins=[],
                            outs=[],
                            sync_info=mybir.SyncInfo(on_wait=[w], on_update=[]),
                        )
                        out.append(ev)
                        n += 1
                    inst.sync_info = mybir.SyncInfo(
                        on_wait=[waits[-1]], on_update=list(si.on_update)
                    )
                out.append(inst)
            blk.instructions = out
    return n


def _ceil_to(x, m):
    return -(-int(x) // m) * m


# feature slot order in the per-side feature tile (each slot = hc-merged
# F cols); chosen so every batched pair-product has affine operand APs.
SLOTS = {"sh": 0, "s1": 1, "c1d": 2, "c2d": 3, "c2t": 4, "s2": 5,
         "c4d2": 6, "s4t": 7, "c4t": 8, "s8t": 9, "c8d": 10,
         "m1": 11, "m2": 12}
NSLOT = 13


def _build_program(D, V, H, JW, ROWS):
    """Uniform SPMD program: one batch per core, ROWS query rows, key
    window JW (padded; mask handles validity)."""
    import contextlib

    import concourse.bass as bass
    import concourse.mybir as mybir
    import concourse.tile as tile

    f32 = mybir.dt.float32
    f16 = mybir.dt.float16
    AF = mybir.ActivationFunctionType
    ALU = mybir.AluOpType

    DC = D // 128
    HC = H // 128
    NCH = _ceil_to(JW, 128) // 128
    lns = [min(128, JW - jc * 128) for jc in range(NCH)]
    P = len(PAIRS)
    RC = ROWS // 128
    NVS = _ceil_to(JW, 128) // 128
    W = FIT_W
    bidx = [p for p in range(P) if abs(BETAS[p]) > 1e-9]

    nc = bass.Bass("TRN2", target_bir_lowering=False)
    d_qT = nc.declare_dram_parameter("qT", [D, ROWS], f16, isOutput=False)
    d_kT = nc.declare_dram_parameter("kT", [D, JW], f16, isOutput=False)
    d_wq = nc.declare_dram_parameter("W_q", [D, H], f16, isOutput=False)
    d_wk = nc.declare_dram_parameter("W_k", [D, H], f16, isOutput=False)
    d_vals = nc.declare_dram_parameter("vals", [NVS * 128, V], f16, isOutput=False)
    d_maskT = nc.declare_dram_parameter("maskT", [128, NCH], f32, isOutput=False)
    d_qcs = nc.declare_dram_parameter("qcs", [128, P * HC], f32, isOutput=False)
    d_wvb = nc.declare_dram_parameter("wvb", [128, max(1, len(bidx)) * HC], f16,
                                      isOutput=False)
    d_out = nc.declare_dram_parameter("out", [ROWS, V], f16, isOutput=True)

    KW = HC * JW    # 576: K part of a feature slot
    QW = HC * ROWS  # 512: Q part
    SW = KW + QW    # merged slot width (K at 0, Q at KW)

    with tile.TileContext(nc) as tc:
        ctx = contextlib.ExitStack()
        with ctx:
            const_pool = ctx.enter_context(tc.tile_pool(name="const", bufs=1))
            in_pool = ctx.enter_context(tc.tile_pool(name="in", bufs=1))
            feat_pool = ctx.enter_context(tc.tile_pool(name="feat", bufs=1))
            qc_pool = ctx.enter_context(tc.tile_pool(name="qc", bufs=1))
            soft_pool = ctx.enter_context(tc.tile_pool(name="soft", bufs=1))
            out_pool = ctx.enter_context(tc.tile_pool(name="outp", bufs=1))
            ppsum = ctx.enter_context(tc.tile_pool(name="pp", bufs=1, space="PSUM"))
            scpsum = ctx.enter_context(tc.tile_pool(name="scp", bufs=2, space="PSUM"))
            opsum = ctx.enter_context(tc.tile_pool(name="op", bufs=2, space="PSUM"))
            smpsum = ctx.enter_context(tc.tile_pool(name="smp", bufs=1, space="PSUM"))

            # Sin table load overlaps the input DMAs
            warm = const_pool.tile([1, 2], f32)
            nc.vector.memset(warm[:], 0.5)
            nc.scalar.activation(warm[0:1, 0:1], warm[0:1, 1:2], AF.Sin)
            ones_f16 = const_pool.tile([128, 1], f16)
            nc.vector.memset(ones_f16[:], 1.0)
            maskT_sb = const_pool.tile([128, NCH], f32)
            qcs_sb = const_pool.tile([128, P * HC], f32)
            wvb_sb = const_pool.tile([128, max(1, len(bidx)) * HC], f16)

            # chunked input DMAs on the two HWDGE queues (SP/ACT);
            # weights/keys first so the k-projection can start early
            qs_ = [nc.sync, nc.sync]
            wk_sb = in_pool.tile([128, DC * H], f16, name="wk")
            kT_sb = in_pool.tile([128, DC * JW], f16, name="kT")
            wq_sb = in_pool.tile([128, DC * H], f16, name="wq")
            qT_sb = in_pool.tile([128, DC * ROWS], f16, name="qT")
            nq = 0
            for dst, dram, cw in ((wk_sb, d_wk, H), (kT_sb, d_kT, JW),
                                  (wq_sb, d_wq, H), (qT_sb, d_qT, ROWS)):
                for h2 in range(DC // 2):
                    qs_[nq % 2].dma_start(
                        out=dst[:, h2 * 2 * cw:(h2 + 1) * 2 * cw].rearrange(
                            "p (dc c) -> p dc c", c=cw),
                        in_=dram[h2 * 256:(h2 + 1) * 256, :].rearrange(
                            "(dc p) c -> p dc c", p=128),
                    )
                    nq += 1
            nc.gpsimd.dma_start(out=maskT_sb[:], in_=d_maskT[:])
            nc.gpsimd.dma_start(out=qcs_sb[:], in_=d_qcs[:])
            nc.gpsimd.dma_start(out=wvb_sb[:], in_=d_wvb[:])
            vals_sb = in_pool.tile([128, NVS * V], f16, name="vals")
            nc.gpsimd.dma_start(
                out=vals_sb[:].rearrange("p (s v) -> p s v", v=V),
                in_=d_vals.rearrange("(s p) v -> p s v", p=128),
            )

            # ---- projections (PE)
            pk = [ppsum.tile([128, JW], f32, tag=f"pk{hc}", name=f"pk{hc}")
                  for hc in range(HC)]
            for hc in range(HC):
                for dc in range(DC):
                    nc.tensor.matmul(
                        pk[hc][:],
                        wk_sb[:, dc * H + hc * 128:dc * H + (hc + 1) * 128],
                        kT_sb[:, dc * JW:(dc + 1) * JW],
                        start=(dc == 0), stop=(dc == DC - 1),
                    )
            pq = ppsum.tile([128, QW], f32, tag="pq", name="pq")
            for hc in range(HC):
                for dc in range(DC):
                    nc.tensor.matmul(
                        pq[:, hc * ROWS:(hc + 1) * ROWS],
                        wq_sb[:, dc * H + hc * 128:dc * H + (hc + 1) * 128],
                        qT_sb[:, dc * ROWS:(dc + 1) * ROWS],
                        start=(dc == 0), stop=(dc == DC - 1),
                    )

            # ---- merged K|Q feature store: slot s = [K (KW) | Q (QW)]
            F = feat_pool.tile([128, NSLOT * SW], f16, name="F")

            def kslot(s, hc=None):
                o = SLOTS[s] * SW
                if hc is None:
                    return F[:, o:o + KW]
                return F[:, o + hc * JW:o + (hc + 1) * JW]

            def qslot(s):
                o = SLOTS[s] * SW + KW
                return F[:, o:o + QW]

            for scale, s in ((0.5 * W, "sh"), (W, "s1"), (2.0 * W, "s2")):
                for hc in range(HC):
                    nc.scalar.activation(
                        kslot(s, hc), pk[hc][:], AF.Sin, scale=scale)
                nc.scalar.activation(qslot(s), pq[:], AF.Sin, scale=scale)
            # Exp table warm-up AFTER the sines (data-dep on the s2 slot so
            # the scheduler cannot hoist it); load overlaps the PE scores
            nc.scalar.activation(warm[0:1, 0:1], qslot("s2")[0:1, 0:1], AF.Exp)

            # ---- DVE product chain, K and Q sides fused per instruction
            fb = F[:]

            def pair_ap(sa, sb):
                return bass.AP(fb.tensor, fb.offset + SLOTS[sa] * SW,
                               [fb.ap[0], [(SLOTS[sb] - SLOTS[sa]) * SW, 2],
                                [1, SW]])

            def sl(s):
                return bass.AP(fb.tensor, fb.offset + SLOTS[s] * SW,
                               [fb.ap[0], [1, SW]])

            # per-side chains: K first (overlaps the Q sines on ACT)
            kp_ = lambda sa, sb: bass.AP(
                fb.tensor, fb.offset + SLOTS[sa] * SW,
                [fb.ap[0], [(SLOTS[sb] - SLOTS[sa]) * SW, 2], [1, KW]])
            qp_ = lambda sa, sb: bass.AP(
                fb.tensor, fb.offset + SLOTS[sa] * SW + KW,
                [fb.ap[0], [(SLOTS[sb] - SLOTS[sa]) * SW, 2], [1, QW]])

            def qsl(s):
                o = SLOTS[s] * SW + KW
                return F[:, o:o + QW]

            def kfull(s):
                return kslot(s)

            def side_chain(pp, kq):
                nc.vector.tensor_mul(pp("c1d", "c2d"), pp("sh", "s1"),
                                     pp("sh", "s1"))
                nc.vector.tensor_scalar_sub(kq("c2t"), kq("c2d"), 0.5)
                nc.vector.tensor_mul(pp("c4d2", "s4t"), pp("c2t", "c2t"),
                                     pp("c2t", "s2"))
                nc.vector.tensor_scalar_sub(kq("c4t"), kq("c4d2"), 0.125)
                nc.vector.tensor_mul(pp("s8t", "c8d"), pp("s4t", "c4t"),
                                     pp("c4t", "c4t"))

            side_chain(kp_, kfull)
            nc.vector.tensor_mul(kslot("m1"), kslot("c2t"), kslot("c4t"))
            nc.vector.tensor_mul(kslot("m2"), kslot("s2"), kslot("s4t"))
            side_chain(qp_, qsl)
            nc.vector.tensor_mul(qp_("m1", "m2"), qp_("c2t", "s2"),
                                 qp_("s4t", "c4t"))

            QSLOT_OF = {"c2s4": "m1", "s2c4": "m2"}
            KSLOT_OF = {"c2c4": "m1", "s2s4": "m2"}

            # ---- Q coeff passes: QC[p,hc] = (alpha_p*|wv|) * Qf  (pure scale;
            # beta side rides the exp bias) -- spread DVE/ACT/Pool
            QC = qc_pool.tile([128, P * HC * ROWS], f16, name="QC")
            for p in range(P):
                qs2 = QSLOT_OF.get(PAIRS[p][0], PAIRS[p][0])
                for hc in range(HC):
                    o = SLOTS[qs2] * SW + KW + hc * ROWS
                    src = F[:, o:o + ROWS]
                    dst = QC[:, (p * HC + hc) * ROWS:(p * HC + hc + 1) * ROWS]
                    col = qcs_sb[:, p * HC + hc:p * HC + hc + 1]
                    if (p * HC + hc) % 3 == 1:
                        nc.scalar.activation(dst, src, AF.Copy, scale=col)
                    else:
                        nc.vector.tensor_scalar_mul(dst, src, col)

            # ---- beta side -> exp bias columns
            biasc = soft_pool.tile([128, NCH], f32, name="biasc")
            nc.vector.tensor_copy(biasc[:], maskT_sb[:])
            misc = smpsum.tile([128, 512], f32, tag="misc", name="misc")
            for jc in range(NCH):
                nmm = 0
                for bi, p in enumerate(bidx):
                    ks = KSLOT_OF.get(PAIRS[p][1], PAIRS[p][1])
                    for hc in range(HC):
                        nc.tensor.matmul(
                            misc[:lns[jc], jc:jc + 1],
                            F[:, SLOTS[ks] * SW + hc * JW + jc * 128:
                               SLOTS[ks] * SW + hc * JW + jc * 128 + lns[jc]],
                            wvb_sb[:, bi * HC + hc:bi * HC + hc + 1],
                            start=(nmm == 0),
                            stop=(nmm == len(bidx) * HC - 1),
                        )
                        nmm += 1
                nc.vector.tensor_add(biasc[:lns[jc], jc:jc + 1],
                                     maskT_sb[:lns[jc], jc:jc + 1],
                                     misc[:lns[jc], jc:jc + 1])

            # ---- scores scT[j,i] and exp
            eT = soft_pool.tile([128, NCH * ROWS], f16, name="eT")
            for jc in range(NCH):
                psc = scpsum.tile([128, ROWS], f32, tag="sc", name=f"psc{jc}")
                nmm = 0
                for p in range(P):
                    ks = KSLOT_OF.get(PAIRS[p][1], PAIRS[p][1])
                    for hc in range(HC):
                        nc.tensor.matmul(
                            psc[:lns[jc], :],
                            F[:, SLOTS[ks] * SW + hc * JW + jc * 128:
                               SLOTS[ks] * SW + hc * JW + jc * 128 + lns[jc]],
                            QC[:, (p * HC + hc) * ROWS:(p * HC + hc + 1) * ROWS],
                            start=(nmm == 0), stop=(nmm == P * HC - 1),
                        )
                        nmm += 1
                nc.scalar.activation(
                    eT[:lns[jc], jc * ROWS:(jc + 1) * ROWS],
                    psc[:lns[jc], :], AF.Exp,
                    bias=biasc[:lns[jc], jc:jc + 1],
                )

            # ---- values + transposed row-sums (shared eT stationaries)
            out_sb = out_pool.tile([128, RC * V], f16, name="osb")
            rinv = soft_pool.tile([128, RC], f32, name="rinv")
            pouts = []
            for rc in range(RC):
                pout = opsum.tile([128, V], f32, tag="po", name=f"pout{rc}")
                pouts.append(pout)
                for jc in range(NCH):
                    nc.tensor.matmul(
                        misc[:, 4 + rc:5 + rc],
                        eT[:lns[jc], jc * ROWS + rc * 128:jc * ROWS + rc * 128 + 128],
                        ones_f16[:lns[jc], 0:1],
                        start=(jc == 0), stop=(jc == NCH - 1),
                    )
                    nc.tensor.matmul(
                        pout[:],
                        eT[:lns[jc], jc * ROWS + rc * 128:jc * ROWS + rc * 128 + 128],
                        vals_sb[:lns[jc], jc * V:(jc + 1) * V],
                        start=(jc == 0), stop=(jc == NCH - 1),
                    )
            nc.vector.reciprocal(rinv[:, 0:RC], misc[:, 4:4 + RC])
            for rc in range(RC):
                if rc == 0:
                    nc.scalar.activation(
                        out_sb[:, rc * V:(rc + 1) * V], pouts[rc][:], AF.Copy,
                        scale=rinv[:, rc:rc + 1])
                else:
                    nc.vector.tensor_scalar_mul(
                        out_sb[:, rc * V:(rc + 1) * V], pouts[rc][:],
                        rinv[:, rc:rc + 1])
                nc.sync.dma_start(
                    out=d_out[rc * 128:(rc + 1) * 128, :],
                    in_=out_sb[:, rc * V:(rc + 1) * V])

    _split_multi_waits(nc)
    return nc


def kernel(queries, keys, values, valid_lens, W_q, W_k, w_v):
    global LAST_RESULT
    _install_axon_profile_hook()
    _patch_tile_drain()
    from concourse.bass_utils import run_bass_kernel_spmd

    import ml_dtypes

    f16 = np.float16
    queries = np.ascontiguousarray(queries, dtype=np.float32)
    keys = np.ascontiguousarray(keys, dtype=np.float32)
    values = np.ascontiguousarray(values, dtype=np.float32)
    W_q = np.ascontiguousarray(W_q, dtype=np.float32)
    W_k = np.ascontiguousarray(W_k, dtype=np.float32)
    w_v = np.ascontiguousarray(w_v, dtype=np.float32)
    vl = np.asarray(valid_lens).astype(np.int64)

    B, Q, D = queries.shape
    KV = keys.shape[1]
    V = values.shape[2]
    H = W_q.shape[1]
    CPB = NCORES // B          # cores per batch
    ROWS = Q // CPB            # query rows per core
    HC = H // 128
    RC = ROWS // 128

    jms = [min(KV, int(v)) for v in vl]
    JW = min(KV, _ceil_to(max(jms), 8))
    NCH = _ceil_to(JW, 128) // 128
    NVS = NCH

    # fold sign(w_v) into the projections (tanh is odd)
    sgn = np.where(w_v >= 0, 1.0, -1.0).astype(np.float32)
    wva = np.abs(w_v)
    Wq_f = (W_q * sgn[None, :]).astype(f16)
    Wk_f = (W_k * sgn[None, :]).astype(f16)

    nc = _build_program(D, V, H, JW, ROWS)

    qcs_cols = []
    for p in range(len(PAIRS)):
        for hc in range(HC):
            qcs_cols.append(ALPHAS[p] * wva[hc * 128:(hc + 1) * 128])
    qcs = np.ascontiguousarray(np.stack(qcs_cols, axis=1), dtype=np.float32)
    bidx = [p for p in range(len(PAIRS)) if abs(BETAS[p]) > 1e-9]
    wvb_cols = []
    for p in bidx:
        for hc in range(HC):
            wvb_cols.append(BETAS[p] * wva[hc * 128:(hc + 1) * 128])
    wvb = np.stack(wvb_cols, axis=1).astype(f16) if wvb_cols else \
        np.zeros((128, 1), f16)
    wvb = np.ascontiguousarray(wvb)

    in_maps = []
    for c in range(NCORES):
        b = c // CPB
        rh = c % CPB
        jm = jms[b]
        qT = np.ascontiguousarray(
            queries[b, rh * ROWS:(rh + 1) * ROWS, :].T.astype(f16))
        kT = np.zeros((D, JW), f16)
        kT[:, :jm] = keys[b, :jm, :].T.astype(f16)
        vals = np.zeros((NVS * 128, V), f16)
        vals[:jm] = values[b, :jm, :].astype(f16)
        j = np.arange(128)
        maskT = np.stack(
            [np.where(jc * 128 + j < jm, 0.0, NEG).astype(np.float32)
             for jc in range(NCH)], axis=1)
        in_maps.append({
            "qT": qT, "kT": np.ascontiguousarray(kT),
            "W_q": Wq_f, "W_k": Wk_f,
            "vals": vals, "maskT": np.ascontiguousarray(maskT),
            "qcs": qcs, "wvb": wvb,
        })

    res = run_bass_kernel_spmd(
        nc, in_maps, core_ids=list(range(NCORES)), trace=TRACE
    )
    LAST_RESULT = res

    out = np.empty((B, Q, V), np.float32)
    for c in range(NCORES):
        b = c // CPB
        rh = c % CPB
        out[b, rh * ROWS:(rh + 1) * ROWS, :] = res.results[c]["out"].astype(
            np.float32)
    return out
